# revision 2
# baseline (speedup 1.0000x reference)
"""GCK 3x3 conv layer (nn_GCK3x3Layer) as a Trainium2 Bass kernel on 8 NeuronCores.

Math: out[o,h,w] = sum_{c,r,s} Weff[o,c,r,s] * x[c,h+r,w+s], where Weff is the
GCK linComb folded back through the +/-1 separable basis (done on host in f64).

Sharding: H split across 8 cores (64 output rows each, 66 input rows with halo).

Device scheme (col-tiled concurrent streams): input rows stored as even/odd
64-partition planes at the same free index.  Per row pair (p normal / p+1
psum-flipped), "main" K=128/M=64 matmuls for two output rows run pairwise
CONCURRENT in opposite PE column groups, and the four K=64 leftover-tap matmuls
fill all four 64x64 array quadrants concurrently: 9 wall slots of 512 cycles
per 2 row pairs (100% PE-array utilization at fp16).

Weight-group schedule (v2): groups of 2 pair-pairs (4 psum banks) run all 12
main slots, then all 6 leftover slots, so the (128,64)<->(64,64) tile-config
switch (measured ~120ns each way; LDWEIGHTS hides fully in steady state but
config changes do not) is paid once per direction per group instead of per
pair-pair.  PSUM: 4 banks/group, 8-bank rotation lets group g+1 accumulate
while group g drains via DVE+ACT copies (fp32->fp16) to a [128, 2048] SBUF
stage, one 4KB-per-partition descriptor DMA per group.

Compute dtype: float16 (10-bit mantissa): measured 2.7e-4 rel err on the graded
seed-0 inputs; fp16 output staging halves output HBM traffic (rel err ~4e-4).

Schedule: early input chunks split across the Scalar+Sync HWDGE rings for fast
first-slot availability; warmup matmuls (DVE-memset scratch, not gpsimd --
gpsimd first-use costs ~5us) lift the PE HAM clock gate (1.2->2.4 GHz) during
the input DMA; outputs + weights on the Scalar ring.
"""

import numpy as np

import concourse.bass as bass
import concourse.mybir as mybir
import concourse.tile as tile
from concourse import bacc
from concourse.bass_utils import run_bass_kernel_spmd

# Problem constants (hardcoded per contract)
C = 64          # input channels
O = 64          # output channels
H = W = 514     # input spatial
HO = WO = 512   # output spatial
NCORES = 8
ROWS_PER_CORE = HO // NCORES          # 64 output rows
PAIRS = ROWS_PER_CORE // 2            # 32 row pairs
JT = ROWS_PER_CORE // 2 + 1           # 33 input row-pair slots (incl. halo)
NT = PAIRS // 2                       # 16 pair-pairs ("t" units, 4 rows each)
GSIZE = 2                             # t units per weight-group (4 psum banks)
NG = NT // GSIZE                      # 8 groups
# Input DMA chunks (row-pair slots, engine): early chunks split across both
# HWDGE rings so slots 0-4 land before the warmup matmuls finish.
XCHUNKS = [("scalar", 0, 3), ("sync", 3, 5), ("sync", 8, 5),
           ("sync", 13, 6), ("sync", 19, 6), ("sync", 25, 8)]
NWARM = 10                            # dummy matmuls to lift the PE HAM clock
                                      # gate (1.2->2.4 GHz) during input DMA

V = np.array([[1.0, 1.0, 1.0], [1.0, -1.0, 1.0], [1.0, 1.0, -1.0]], dtype=np.float64)

MM_DT = mybir.dt.float16   # matmul operand dtype
OUT_DT = mybir.dt.float16  # output staging dtype (cast back to f32 on host)


def _fold_weights(linCombs: np.ndarray) -> np.ndarray:
    """linCombs (O, C*9) -> effective conv kernels Weff (O, C, 3, 3), f64."""
    L = linCombs.astype(np.float64).reshape(O, C, 3, 3)  # k = c*9 + 3i + j
    return np.einsum("ocij,ir,js->ocrs", L, V, V)


def _build_block_weights(Weff: np.ndarray) -> np.ndarray:
    """Weights for the col-tiled scheme, returned as [128, 9, 64] (k, idx, mu).

    idx 0..2  (s): K0=Wt(0,s), K1=Wt(1,s)  -- mains for EVEN output rows (rhs slot p)
    idx 3..5  (s): K0=Wt(1,s), K1=Wt(2,s)  -- mains for ODD  output rows (rhs slot p+1)
    idx 6..8  (s): K0=Wt(2,s) (leftover r2, even-plane rhs, partitions 0-63)
                   K1=Wt(0,s) (leftover r0, odd-plane rhs,  partitions 64-127)
    """
    Wt = {(r, s): Weff[:, :, r, s].T for r in range(3) for s in range(3)}  # [c, o]
    mats = np.zeros((9, 128, 64), dtype=np.float64)
    for s in range(3):
        mats[s, 0:64] = Wt[(0, s)]
        mats[s, 64:128] = Wt[(1, s)]
        mats[3 + s, 0:64] = Wt[(1, s)]
        mats[3 + s, 64:128] = Wt[(2, s)]
        mats[6 + s, 0:64] = Wt[(2, s)]
        mats[6 + s, 64:128] = Wt[(0, s)]
    return np.ascontiguousarray(mats.transpose(1, 0, 2).astype(np.float16))


def _build_program():
    nc = bacc.Bacc(None, target_bir_lowering=False, enable_partition_id=False)
    xin = nc.declare_dram_parameter("xin", [128, JT, W], MM_DT, isOutput=False)
    wts = nc.declare_dram_parameter("wts", [128, 9, 64], MM_DT, isOutput=False)
    yout = nc.declare_dram_parameter(
        "yout", [NG, 128, 2 * GSIZE * WO], OUT_DT, isOutput=True
    )

    with tile.TileContext(nc) as tc:
        with (
            tc.tile_pool(name="wpool", bufs=1) as wpool,
            tc.tile_pool(name="xpool", bufs=1) as xpool,
            tc.tile_pool(name="opool", bufs=3) as opool,
            tc.tile_pool(name="pspool", bufs=8, space="PSUM") as pspool,
        ):
            wt = wpool.tile([128, 9, 64], MM_DT)
            nc.scalar.dma_start(wt[:], wts[:])

            # PE warmup on a DVE-zeroed scratch tile while the input DMAs are
            # in flight, so the HAM clock gate is at 8/8 (2.4 GHz) when the
            # real matmuls start.  wps shares the psum rotation (slot 0 is
            # reused by a later group once the warmups are done).
            warm = wpool.tile([128, WO], mybir.dt.bfloat16, name="warm")
            nc.vector.memset(warm[:], 0.0)
            # preload the ACT Copy table off the critical path (first
            # activation-table use costs ~2.7us)
            actw = wpool.tile([128, 16], mybir.dt.float32, name="actw")
            nc.scalar.copy(actw[:], warm[:, 0:16])
            wps = pspool.tile([128, WO], mybir.dt.float32, tag="ps", name="wps")
            for _ in range(NWARM):
                nc.tensor.matmul(
                    wps[:], warm[:, :128], warm[:], start=True, stop=True
                )

            xgs = []
            for eng, start, n in XCHUNKS:
                xt = xpool.tile([128, n * W], MM_DT, tag=f"xt{start}", name=f"xt{start}")
                getattr(nc, eng).dma_start(xt[:], xin[:, start : start + n, :])
                xgs.append((start, n, xt))

            def rhs(j, s):
                for start, n, xt in reversed(xgs):
                    if j >= start:
                        off = j - start
                        return xt[:, off * W + s : off * W + s + WO]
                raise AssertionError(j)

            for g in range(NG):
                ts = [GSIZE * g + i for i in range(GSIZE)]
                # psum tiles: ps[i][0] covers rows (4t, 4t+1) normal,
                # ps[i][1] covers rows (4t+2, 4t+3) psum-flipped.
                ps = [
                    [
                        pspool.tile([128, WO], mybir.dt.float32, tag="ps",
                                    name=f"ps{t}_{v}")
                        for v in range(2)
                    ]
                    for t in ts
                ]
                # mains: 6 slots per t, uniform (128,64) tile config;
                # LDWEIGHTS hides under the opposite col-group's stream.
                for i, t in enumerate(ts):
                    p0, p1 = 2 * t, 2 * t + 1
                    for s in range(3):
                        nc.tensor.matmul(ps[i][0][0:64, :], wt[:, s, :],
                                         rhs(p0, s), start=(s == 0), stop=False)
                        nc.tensor.matmul(ps[i][0][64:128, :], wt[:, 3 + s, :],
                                         rhs(p0 + 1, s), start=(s == 0), stop=False)
                    for s in range(3):
                        nc.tensor.matmul(ps[i][1][64:128, :], wt[:, s, :],
                                         rhs(p1, s), start=(s == 0), stop=False)
                        nc.tensor.matmul(ps[i][1][0:64, :], wt[:, 3 + s, :],
                                         rhs(p1 + 1, s), start=(s == 0), stop=False)
                # leftovers: 3 slots per t, all four (64,64) quadrants
                # concurrent; one tile-config switch per group each way.
                for s in range(3):
                    for i, t in enumerate(ts):
                        p0, p1 = 2 * t, 2 * t + 1
                        e0 = rhs(p0 + 1, s)  # even half -> partitions 0-63
                        o0 = rhs(p0, s)      # odd half  -> partitions 64-127
                        e1 = rhs(p1 + 1, s)
                        o1 = rhs(p1, s)
                        nc.tensor.matmul(ps[i][0][0:64, :], wt[0:64, 6 + s, :],
                                         e0[0:64, :], start=False, stop=(s == 2))
                        nc.tensor.matmul(ps[i][0][64:128, :], wt[64:128, 6 + s, :],
                                         o0[64:128, :], start=False, stop=(s == 2))
                        nc.tensor.matmul(ps[i][1][64:128, :], wt[0:64, 6 + s, :],
                                         e1[0:64, :], start=False, stop=(s == 2))
                        nc.tensor.matmul(ps[i][1][0:64, :], wt[64:128, 6 + s, :],
                                         o1[64:128, :], start=False, stop=(s == 2))
                # drain: fp32->fp16 copies split across DVE and ACT, then one
                # 4KB-per-partition-descriptor DMA (last group: split DMAs as
                # each copy lands, to shorten the tail).
                ot = opool.tile([128, 2 * GSIZE * WO], OUT_DT, tag="ot", name=f"ot{g}")
                last = g == NG - 1
                for i in range(GSIZE):
                    cv = nc.vector.tensor_copy(
                        ot[:, (2 * i) * WO : (2 * i + 1) * WO], ps[i][0][:])
                    cs = nc.scalar.copy(
                        ot[:, (2 * i + 1) * WO : (2 * i + 2) * WO], ps[i][1][:])
                    if last:
                        nc.sync.dma_start(
                            yout[g][:, (2 * i) * WO : (2 * i + 1) * WO],
                            ot[:, (2 * i) * WO : (2 * i + 1) * WO])
                        nc.scalar.dma_start(
                            yout[g][:, (2 * i + 1) * WO : (2 * i + 2) * WO],
                            ot[:, (2 * i + 1) * WO : (2 * i + 2) * WO])
                if not last:
                    nc.scalar.dma_start(yout[g], ot[:])

    nc.compile()
    return nc


_NC_CACHE = None


def _prep_inputs(x0: np.ndarray, linCombs: np.ndarray):
    """x0 [C,H,W] f32, linCombs [O, C*9] -> per-core in_maps."""
    Weff = _fold_weights(linCombs)
    wts_h = _build_block_weights(Weff)
    in_maps = []
    for core in range(NCORES):
        r0 = core * ROWS_PER_CORE
        ev = x0[:, r0:r0 + 2 * JT:2, :]       # [64, 33, W] even local rows
        od = x0[:, r0 + 1:r0 + 2 * JT:2, :]   # [64, 33, W] odd local rows
        P = np.ascontiguousarray(
            np.concatenate([ev, od], axis=0).astype(np.float16))
        in_maps.append({"xin": P, "wts": wts_h})
    return in_maps


def _decode_output(results) -> np.ndarray:
    out = np.empty((1, O, HO, WO), dtype=np.float32)
    for core in range(NCORES):
        y = np.asarray(results[core]["yout"], dtype=np.float32)
        # [NG, 128, 2*GSIZE*WO] -> [g, par(2), o(64), tloc, v, w]
        y = y.reshape(NG, 2, O, GSIZE, 2, WO)
        r0 = core * ROWS_PER_CORE
        for tloc in range(GSIZE):
            for v in range(2):
                for par in range(2):
                    row_off = 4 * tloc + 2 * v + (par if v == 0 else 1 - par)
                    out[0, :, r0 + row_off : r0 + ROWS_PER_CORE : 4 * GSIZE, :] = (
                        y[:, par, :, tloc, v, :].transpose(1, 0, 2))
    return out


def kernel(input: np.ndarray, linCombs: np.ndarray) -> np.ndarray:
    global _NC_CACHE
    x = np.ascontiguousarray(np.asarray(input, dtype=np.float32))
    L = np.asarray(linCombs, dtype=np.float32)
    assert x.shape == (1, C, H, W), x.shape

    in_maps = _prep_inputs(x[0], L)
    if _NC_CACHE is None:
        _NC_CACHE = _build_program()
    res = run_bass_kernel_spmd(_NC_CACHE, in_maps, list(range(NCORES)))
    return _decode_output(res.results)


# revision 5
# speedup vs baseline: 1.0403x; 1.0403x over previous
"""GCK 3x3 conv layer (nn_GCK3x3Layer) as a Trainium2 Bass kernel on 8 NeuronCores.

Math: out[o,h,w] = sum_{c,r,s} Weff[o,c,r,s] * x[c,h+r,w+s], where Weff is the
GCK linComb folded back through the +/-1 separable basis (done on host in f64).

Sharding: H split across 8 cores (64 output rows each, 66 input rows with halo).

Device scheme (col-tiled concurrent streams): input rows stored as even/odd
64-partition planes at the same free index.  Per row pair (p normal / p+1
psum-flipped), "main" K=128/M=64 matmuls for two output rows run pairwise
CONCURRENT in opposite PE column groups, and the four K=64 leftover-tap matmuls
fill all four 64x64 array quadrants concurrently: 9 wall slots of 512 cycles
per 2 row pairs (100% PE-array utilization at fp16).

Weight-group schedule (v2): groups of 2 pair-pairs (4 psum banks) run all 12
main slots, then all 6 leftover slots, so the (128,64)<->(64,64) tile-config
switch (measured ~120ns each way; LDWEIGHTS hides fully in steady state but
config changes do not) is paid once per direction per group instead of per
pair-pair.  PSUM: 4 banks/group, 8-bank rotation lets group g+1 accumulate
while group g drains via DVE+ACT copies (fp32->fp16) to a [128, 2048] SBUF
stage, one 4KB-per-partition descriptor DMA per group.

Compute dtype: float16 (10-bit mantissa): measured 2.7e-4 rel err on the graded
seed-0 inputs; fp16 output staging halves output HBM traffic (rel err ~4e-4).

Schedule: early input chunks split across the Scalar+Sync HWDGE rings for fast
first-slot availability; warmup matmuls (DVE-memset scratch, not gpsimd --
gpsimd first-use costs ~5us) lift the PE HAM clock gate (1.2->2.4 GHz) during
the input DMA; outputs + weights on the Scalar ring.
"""

import numpy as np

import concourse.bass as bass
import concourse.mybir as mybir
import concourse.tile as tile
from concourse import bacc
from concourse.bass_utils import run_bass_kernel_spmd

# Problem constants (hardcoded per contract)
C = 64          # input channels
O = 64          # output channels
H = W = 514     # input spatial
HO = WO = 512   # output spatial
NCORES = 8
ROWS_PER_CORE = HO // NCORES          # 64 output rows
PAIRS = ROWS_PER_CORE // 2            # 32 row pairs
JT = ROWS_PER_CORE // 2 + 1           # 33 input row-pair slots (incl. halo)
NT = PAIRS // 2                       # 16 pair-pairs ("t" units, 4 rows each)
GSIZE = 2                             # t units per weight-group (4 psum banks)
NG = NT // GSIZE                      # 8 groups
# Input DMA chunks (row-pair slots): ALL on the sync ring in ascending order
# with ramped sizes -- SDMA engines drain at packet granularity, so a small
# urgent transfer issued behind (or beside, on the other ring) a bulk queue
# gets starved; the first slots must be the first packets on the ring.
XGS = [1, 1, 2, 3, 4, 5, 5, 6, 6]     # sums to 33
XGO = [sum(XGS[:i]) for i in range(len(XGS))]
NWARM = 5                             # dummy matmuls to lift the PE HAM clock
                                      # gate (1.2->2.4 GHz) during input DMA

V = np.array([[1.0, 1.0, 1.0], [1.0, -1.0, 1.0], [1.0, 1.0, -1.0]], dtype=np.float64)

MM_DT = mybir.dt.float16   # matmul operand dtype
OUT_DT = mybir.dt.float16  # output staging dtype (cast back to f32 on host)


def _fold_weights(linCombs: np.ndarray) -> np.ndarray:
    """linCombs (O, C*9) -> effective conv kernels Weff (O, C, 3, 3), f64."""
    L = linCombs.astype(np.float64).reshape(O, C, 3, 3)  # k = c*9 + 3i + j
    return np.einsum("ocij,ir,js->ocrs", L, V, V)


def _build_block_weights(Weff: np.ndarray) -> np.ndarray:
    """Weights for the col-tiled scheme, returned as [128, 9, 64] (k, idx, mu).

    idx 0..2  (s): K0=Wt(0,s), K1=Wt(1,s)  -- mains for EVEN output rows (rhs slot p)
    idx 3..5  (s): K0=Wt(1,s), K1=Wt(2,s)  -- mains for ODD  output rows (rhs slot p+1)
    idx 6..8  (s): K0=Wt(2,s) (leftover r2, even-plane rhs, partitions 0-63)
                   K1=Wt(0,s) (leftover r0, odd-plane rhs,  partitions 64-127)
    """
    Wt = {(r, s): Weff[:, :, r, s].T for r in range(3) for s in range(3)}  # [c, o]
    mats = np.zeros((9, 128, 64), dtype=np.float64)
    for s in range(3):
        mats[s, 0:64] = Wt[(0, s)]
        mats[s, 64:128] = Wt[(1, s)]
        mats[3 + s, 0:64] = Wt[(1, s)]
        mats[3 + s, 64:128] = Wt[(2, s)]
        mats[6 + s, 0:64] = Wt[(2, s)]
        mats[6 + s, 64:128] = Wt[(0, s)]
    return np.ascontiguousarray(mats.transpose(1, 0, 2).astype(np.float16))


def _build_program():
    nc = bacc.Bacc(None, target_bir_lowering=False, enable_partition_id=False)
    xin = nc.declare_dram_parameter("xin", [128, JT, W], MM_DT, isOutput=False)
    wts = nc.declare_dram_parameter("wts", [128, 9, 64], MM_DT, isOutput=False)
    yout = nc.declare_dram_parameter(
        "yout", [NG, 128, 2 * GSIZE * WO], OUT_DT, isOutput=True
    )

    with tile.TileContext(nc) as tc:
        with (
            tc.tile_pool(name="wpool", bufs=1) as wpool,
            tc.tile_pool(name="xpool", bufs=1) as xpool,
            tc.tile_pool(name="opool", bufs=3) as opool,
            tc.tile_pool(name="pspool", bufs=8, space="PSUM") as pspool,
        ):
            wt = wpool.tile([128, 9, 64], MM_DT)
            nc.scalar.dma_start(wt[:], wts[:])

            # PE warmup on a gpsimd-zeroed scratch tile (gpsimd picks up user
            # work earliest, ~6us; DVE's first op lands ~1.5us later) while
            # the input DMAs are in flight, so the HAM clock gate is ramping
            # toward 8/8 (2.4 GHz) when the real matmuls start.  wps shares
            # the psum rotation (slot 0 is reused by a later group).
            warm = wpool.tile([128, WO], mybir.dt.bfloat16, name="warm")
            nc.gpsimd.memset(warm[:], 0.0)
            # preload the ACT table off the critical path (first use ~1.3us);
            # placed after the wts trigger so no DMA queues behind it.
            actw = wpool.tile([128, 16], mybir.dt.float32, name="actw")
            nc.scalar.copy(actw[:], warm[:, 0:16])
            wps = pspool.tile([128, WO], mybir.dt.float32, tag="ps", name="wps")
            for _ in range(NWARM):
                nc.tensor.matmul(
                    wps[:], warm[:, :128], warm[:], start=True, stop=True
                )

            xgs = []
            for gx, n in enumerate(XGS):
                xt = xpool.tile([128, n * W], MM_DT, tag=f"xt{gx}", name=f"xt{gx}")
                nc.sync.dma_start(xt[:], xin[:, XGO[gx] : XGO[gx] + n, :])
                xgs.append((XGO[gx], n, xt))

            def rhs(j, s):
                for start, n, xt in reversed(xgs):
                    if j >= start:
                        off = j - start
                        return xt[:, off * W + s : off * W + s + WO]
                raise AssertionError(j)

            for g in range(NG):
                ts = [GSIZE * g + i for i in range(GSIZE)]
                # psum tiles: ps[i][0] covers rows (4t, 4t+1) normal,
                # ps[i][1] covers rows (4t+2, 4t+3) psum-flipped.
                ps = [
                    [
                        pspool.tile([128, WO], mybir.dt.float32, tag="ps",
                                    name=f"ps{t}_{v}")
                        for v in range(2)
                    ]
                    for t in ts
                ]
                # ABBA phase order: even groups run mains -> leftovers, odd
                # groups leftovers -> mains, so the (128,64)<->(64,64) PE
                # tile-config switch (~120ns) is paid once per group, not
                # twice.  start=True goes on each psum half's first matmul,
                # stop=True on its last, whichever phase that lands in.
                lo_first = g % 2 == 1

                def mains(first, last):
                    # 6 slots per t, uniform (128,64) tile config; LDWEIGHTS
                    # hides under the opposite col-group's stream.
                    for i, t in enumerate(ts):
                        p0, p1 = 2 * t, 2 * t + 1
                        for s in range(3):
                            st, sp = first and s == 0, last and s == 2
                            nc.tensor.matmul(ps[i][0][0:64, :], wt[:, s, :],
                                             rhs(p0, s), start=st, stop=sp)
                            nc.tensor.matmul(ps[i][0][64:128, :], wt[:, 3 + s, :],
                                             rhs(p0 + 1, s), start=st, stop=sp)
                        for s in range(3):
                            st, sp = first and s == 0, last and s == 2
                            nc.tensor.matmul(ps[i][1][64:128, :], wt[:, s, :],
                                             rhs(p1, s), start=st, stop=sp)
                            nc.tensor.matmul(ps[i][1][0:64, :], wt[:, 3 + s, :],
                                             rhs(p1 + 1, s), start=st, stop=sp)

                def leftovers(first, last):
                    # 3 slots per t, all four (64,64) quadrants concurrent.
                    for s in range(3):
                        st, sp = first and s == 0, last and s == 2
                        for i, t in enumerate(ts):
                            p0, p1 = 2 * t, 2 * t + 1
                            e0 = rhs(p0 + 1, s)  # even half -> partitions 0-63
                            o0 = rhs(p0, s)      # odd half  -> partitions 64-127
                            e1 = rhs(p1 + 1, s)
                            o1 = rhs(p1, s)
                            nc.tensor.matmul(ps[i][0][0:64, :], wt[0:64, 6 + s, :],
                                             e0[0:64, :], start=st, stop=sp)
                            nc.tensor.matmul(ps[i][0][64:128, :], wt[64:128, 6 + s, :],
                                             o0[64:128, :], start=st, stop=sp)
                            nc.tensor.matmul(ps[i][1][64:128, :], wt[0:64, 6 + s, :],
                                             e1[0:64, :], start=st, stop=sp)
                            nc.tensor.matmul(ps[i][1][0:64, :], wt[64:128, 6 + s, :],
                                             o1[64:128, :], start=st, stop=sp)

                if lo_first:
                    leftovers(True, False)
                    mains(False, True)
                else:
                    mains(True, False)
                    leftovers(False, True)
                # drain: fp32->fp16 copies split across DVE and ACT, then one
                # 4KB-per-partition-descriptor DMA (last group: split DMAs as
                # each copy lands, to shorten the tail).
                ot = opool.tile([128, 2 * GSIZE * WO], OUT_DT, tag="ot", name=f"ot{g}")
                last = g == NG - 1
                for i in range(GSIZE):
                    cv = nc.vector.tensor_copy(
                        ot[:, (2 * i) * WO : (2 * i + 1) * WO], ps[i][0][:])
                    cs = nc.scalar.copy(
                        ot[:, (2 * i + 1) * WO : (2 * i + 2) * WO], ps[i][1][:])
                    if last:
                        nc.sync.dma_start(
                            yout[g][:, (2 * i) * WO : (2 * i + 1) * WO],
                            ot[:, (2 * i) * WO : (2 * i + 1) * WO])
                        nc.scalar.dma_start(
                            yout[g][:, (2 * i + 1) * WO : (2 * i + 2) * WO],
                            ot[:, (2 * i + 1) * WO : (2 * i + 2) * WO])
                if not last:
                    nc.scalar.dma_start(yout[g], ot[:])

    nc.compile()
    return nc


_NC_CACHE = None


def _prep_inputs(x0: np.ndarray, linCombs: np.ndarray):
    """x0 [C,H,W] f32, linCombs [O, C*9] -> per-core in_maps."""
    Weff = _fold_weights(linCombs)
    wts_h = _build_block_weights(Weff)
    in_maps = []
    for core in range(NCORES):
        r0 = core * ROWS_PER_CORE
        ev = x0[:, r0:r0 + 2 * JT:2, :]       # [64, 33, W] even local rows
        od = x0[:, r0 + 1:r0 + 2 * JT:2, :]   # [64, 33, W] odd local rows
        P = np.ascontiguousarray(
            np.concatenate([ev, od], axis=0).astype(np.float16))
        in_maps.append({"xin": P, "wts": wts_h})
    return in_maps


def _decode_output(results) -> np.ndarray:
    out = np.empty((1, O, HO, WO), dtype=np.float32)
    for core in range(NCORES):
        y = np.asarray(results[core]["yout"], dtype=np.float32)
        # [NG, 128, 2*GSIZE*WO] -> [g, par(2), o(64), tloc, v, w]
        y = y.reshape(NG, 2, O, GSIZE, 2, WO)
        r0 = core * ROWS_PER_CORE
        for tloc in range(GSIZE):
            for v in range(2):
                for par in range(2):
                    row_off = 4 * tloc + 2 * v + (par if v == 0 else 1 - par)
                    out[0, :, r0 + row_off : r0 + ROWS_PER_CORE : 4 * GSIZE, :] = (
                        y[:, par, :, tloc, v, :].transpose(1, 0, 2))
    return out


def kernel(input: np.ndarray, linCombs: np.ndarray) -> np.ndarray:
    global _NC_CACHE
    x = np.ascontiguousarray(np.asarray(input, dtype=np.float32))
    L = np.asarray(linCombs, dtype=np.float32)
    assert x.shape == (1, C, H, W), x.shape

    in_maps = _prep_inputs(x[0], L)
    if _NC_CACHE is None:
        _NC_CACHE = _build_program()
    res = run_bass_kernel_spmd(_NC_CACHE, in_maps, list(range(NCORES)))
    return _decode_output(res.results)


# revision 9
# speedup vs baseline: 1.1074x; 1.0644x over previous
"""GCK 3x3 conv layer (nn_GCK3x3Layer) as a Trainium2 Bass kernel on 8 NeuronCores.

Math: out[o,h,w] = sum_{c,r,s} Weff[o,c,r,s] * x[c,h+r,w+s], where Weff is the
GCK linComb folded back through the +/-1 separable basis (done on host in f64).

Sharding: H split across 8 cores (64 output rows each, 66 input rows with halo).

Device scheme (col-tiled concurrent streams): input rows stored as even/odd
64-partition planes at the same free index.  Per row pair (p normal / p+1
psum-flipped), "main" K=128/M=64 matmuls for two output rows run pairwise
CONCURRENT in opposite PE column groups, and the four K=64 leftover-tap matmuls
fill all four 64x64 array quadrants concurrently: 9 wall slots of 512 cycles
per 2 row pairs (100% PE-array utilization at fp16).

Weight-group schedule (v2): groups of 2 pair-pairs (4 psum banks) run all 12
main slots, then all 6 leftover slots, so the (128,64)<->(64,64) tile-config
switch (measured ~120ns each way; LDWEIGHTS hides fully in steady state but
config changes do not) is paid once per direction per group instead of per
pair-pair.  PSUM: 4 banks/group, 8-bank rotation lets group g+1 accumulate
while group g drains via DVE+ACT copies (fp32->fp16) to a [128, 2048] SBUF
stage, one 4KB-per-partition descriptor DMA per group.

Compute dtype: float16 (10-bit mantissa): measured 2.7e-4 rel err on the graded
seed-0 inputs; fp16 output staging halves output HBM traffic (rel err ~4e-4).

Schedule: early input chunks split across the Scalar+Sync HWDGE rings for fast
first-slot availability; warmup matmuls (DVE-memset scratch, not gpsimd --
gpsimd first-use costs ~5us) lift the PE HAM clock gate (1.2->2.4 GHz) during
the input DMA; outputs + weights on the Scalar ring.
"""

import numpy as np

import concourse.bass as bass
import concourse.mybir as mybir
import concourse.tile as tile
from concourse import bacc
from concourse.bass_utils import run_bass_kernel_spmd

# Problem constants (hardcoded per contract)
C = 64          # input channels
O = 64          # output channels
H = W = 514     # input spatial
HO = WO = 512   # output spatial
NCORES = 8
ROWS_PER_CORE = HO // NCORES          # 64 output rows
PAIRS = ROWS_PER_CORE // 2            # 32 row pairs
JT = ROWS_PER_CORE // 2 + 1           # 33 input row-pair slots (incl. halo)
NT = PAIRS // 2                       # 16 pair-pairs ("t" units, 4 rows each)
GSIZE = 2                             # t units per weight-group (4 psum banks)
NG = NT // GSIZE                      # 8 groups
# Input DMA chunks (row-pair slots): ALL on the sync ring in ascending order
# with ramped sizes -- SDMA engines drain at packet granularity, so a small
# urgent transfer issued behind (or beside, on the other ring) a bulk queue
# gets starved; the first slots must be the first packets on the ring.
XGS = [3, 3, 4, 5, 6, 6, 6]           # sums to 33
XGO = [sum(XGS[:i]) for i in range(len(XGS))]
NWARM = 7                             # dummy matmuls to lift the PE HAM clock
                                      # gate (1.2->2.4 GHz) during input DMA

V = np.array([[1.0, 1.0, 1.0], [1.0, -1.0, 1.0], [1.0, 1.0, -1.0]], dtype=np.float64)

MM_DT = mybir.dt.float16   # matmul operand dtype
OUT_DT = mybir.dt.float16  # output staging dtype (cast back to f32 on host)


def _fold_weights(linCombs: np.ndarray) -> np.ndarray:
    """linCombs (O, C*9) -> effective conv kernels Weff (O, C, 3, 3), f64."""
    L = linCombs.astype(np.float64).reshape(O, C, 3, 3)  # k = c*9 + 3i + j
    return np.einsum("ocij,ir,js->ocrs", L, V, V)


def _build_block_weights(Weff: np.ndarray) -> np.ndarray:
    """Weights for the col-tiled scheme, returned as [128, 9, 64] (k, idx, mu).

    idx 0..2  (s): K0=Wt(0,s), K1=Wt(1,s)  -- mains for EVEN output rows (rhs slot p)
    idx 3..5  (s): K0=Wt(1,s), K1=Wt(2,s)  -- mains for ODD  output rows (rhs slot p+1)
    idx 6..8  (s): K0=Wt(2,s) (leftover r2, even-plane rhs, partitions 0-63)
                   K1=Wt(0,s) (leftover r0, odd-plane rhs,  partitions 64-127)
    """
    Wt = {(r, s): Weff[:, :, r, s].T for r in range(3) for s in range(3)}  # [c, o]
    mats = np.zeros((9, 128, 64), dtype=np.float64)
    for s in range(3):
        mats[s, 0:64] = Wt[(0, s)]
        mats[s, 64:128] = Wt[(1, s)]
        mats[3 + s, 0:64] = Wt[(1, s)]
        mats[3 + s, 64:128] = Wt[(2, s)]
        mats[6 + s, 0:64] = Wt[(2, s)]
        mats[6 + s, 64:128] = Wt[(0, s)]
    return np.ascontiguousarray(mats.transpose(1, 0, 2).astype(np.float16))


def _build_program():
    nc = bacc.Bacc(None, target_bir_lowering=False, enable_partition_id=False)
    xin = nc.declare_dram_parameter("xin", [128, JT, W], MM_DT, isOutput=False)
    wts = nc.declare_dram_parameter("wts", [128, 9, 64], MM_DT, isOutput=False)
    yout = nc.declare_dram_parameter(
        "yout", [NG, 128, 2 * GSIZE * WO], OUT_DT, isOutput=True
    )

    with tile.TileContext(nc) as tc:
        with (
            tc.tile_pool(name="wpool", bufs=1) as wpool,
            tc.tile_pool(name="xpool", bufs=1) as xpool,
            tc.tile_pool(name="opool", bufs=3) as opool,
            tc.tile_pool(name="pspool", bufs=8, space="PSUM") as pspool,
        ):
            wt = wpool.tile([128, 9, 64], MM_DT)
            nc.scalar.dma_start(wt[:], wts[:])

            # PE warmup matmuls on a gpsimd-zeroed scratch tile (gpsimd picks
            # up user work earliest, ~6us; reading an unwritten tile is
            # rejected by Tile), so the HAM clock gate is ramping toward 8/8
            # (2.4 GHz) during the input DMA window.  wps shares the psum
            # rotation (slot 0 is reused by a later group once warmups done).
            warm = wpool.tile([128, WO], mybir.dt.bfloat16, name="warm")
            nc.gpsimd.memset(warm[:], 0.0)
            # preload the ACT table off the critical path (first use ~1.3us);
            # placed after the wts trigger so no DMA queues behind it.
            actw = wpool.tile([128, 16], mybir.dt.float32, name="actw")
            nc.scalar.copy(actw[:], warm[:, 0:16])
            wps = pspool.tile([128, WO], mybir.dt.float32, tag="ps", name="wps")
            for _ in range(NWARM):
                nc.tensor.matmul(
                    wps[:], warm[:, :128], warm[:], start=True, stop=True
                )

            xgs = []
            for gx, n in enumerate(XGS):
                xt = xpool.tile([128, n * W], MM_DT, tag=f"xt{gx}", name=f"xt{gx}")
                nc.sync.dma_start(xt[:], xin[:, XGO[gx] : XGO[gx] + n, :])
                xgs.append((XGO[gx], n, xt))

            def rhs(j, s):
                for start, n, xt in reversed(xgs):
                    if j >= start:
                        off = j - start
                        return xt[:, off * W + s : off * W + s + WO]
                raise AssertionError(j)

            for g in range(NG):
                ts = [GSIZE * g + i for i in range(GSIZE)]
                # psum tiles: ps[i][0] covers rows (4t, 4t+1) normal,
                # ps[i][1] covers rows (4t+2, 4t+3) psum-flipped.
                ps = [
                    [
                        pspool.tile([128, WO], mybir.dt.float32, tag="ps",
                                    name=f"ps{t}_{v}")
                        for v in range(2)
                    ]
                    for t in ts
                ]
                # ABBA phase order: even groups run mains -> leftovers, odd
                # groups leftovers -> mains, so the (128,64)<->(64,64) PE
                # tile-config switch (~120ns) is paid once per group, not
                # twice.  start=True goes on each psum half's first matmul,
                # stop=True on its last, whichever phase that lands in.
                lo_first = g % 2 == 1

                def mains(first, last):
                    # 6 slots per t, uniform (128,64) tile config; LDWEIGHTS
                    # hides under the opposite col-group's stream.
                    for i, t in enumerate(ts):
                        p0, p1 = 2 * t, 2 * t + 1
                        for s in range(3):
                            st, sp = first and s == 0, last and s == 2
                            nc.tensor.matmul(ps[i][0][0:64, :], wt[:, s, :],
                                             rhs(p0, s), start=st, stop=sp)
                            nc.tensor.matmul(ps[i][0][64:128, :], wt[:, 3 + s, :],
                                             rhs(p0 + 1, s), start=st, stop=sp)
                        for s in range(3):
                            st, sp = first and s == 0, last and s == 2
                            nc.tensor.matmul(ps[i][1][64:128, :], wt[:, s, :],
                                             rhs(p1, s), start=st, stop=sp)
                            nc.tensor.matmul(ps[i][1][0:64, :], wt[:, 3 + s, :],
                                             rhs(p1 + 1, s), start=st, stop=sp)

                def leftovers(first, last):
                    # 3 slots per t, all four (64,64) quadrants concurrent.
                    for s in range(3):
                        st, sp = first and s == 0, last and s == 2
                        for i, t in enumerate(ts):
                            p0, p1 = 2 * t, 2 * t + 1
                            e0 = rhs(p0 + 1, s)  # even half -> partitions 0-63
                            o0 = rhs(p0, s)      # odd half  -> partitions 64-127
                            e1 = rhs(p1 + 1, s)
                            o1 = rhs(p1, s)
                            nc.tensor.matmul(ps[i][0][0:64, :], wt[0:64, 6 + s, :],
                                             e0[0:64, :], start=st, stop=sp)
                            nc.tensor.matmul(ps[i][0][64:128, :], wt[64:128, 6 + s, :],
                                             o0[64:128, :], start=st, stop=sp)
                            nc.tensor.matmul(ps[i][1][64:128, :], wt[0:64, 6 + s, :],
                                             e1[0:64, :], start=st, stop=sp)
                            nc.tensor.matmul(ps[i][1][0:64, :], wt[64:128, 6 + s, :],
                                             o1[64:128, :], start=st, stop=sp)

                if lo_first:
                    leftovers(True, False)
                    mains(False, True)
                else:
                    mains(True, False)
                    leftovers(False, True)
                # drain: fp32->fp16 copies split across DVE and ACT, then one
                # 4KB-per-partition-descriptor DMA (last group: split DMAs as
                # each copy lands, to shorten the tail).
                ot = opool.tile([128, 2 * GSIZE * WO], OUT_DT, tag="ot", name=f"ot{g}")
                last = g == NG - 1
                for i in range(GSIZE):
                    cv = nc.vector.tensor_copy(
                        ot[:, (2 * i) * WO : (2 * i + 1) * WO], ps[i][0][:])
                    cs = nc.scalar.copy(
                        ot[:, (2 * i + 1) * WO : (2 * i + 2) * WO], ps[i][1][:])
                    if last:
                        nc.sync.dma_start(
                            yout[g][:, (2 * i) * WO : (2 * i + 1) * WO],
                            ot[:, (2 * i) * WO : (2 * i + 1) * WO])
                        nc.scalar.dma_start(
                            yout[g][:, (2 * i + 1) * WO : (2 * i + 2) * WO],
                            ot[:, (2 * i + 1) * WO : (2 * i + 2) * WO])
                if not last:
                    nc.scalar.dma_start(yout[g], ot[:])

    nc.compile()
    return nc


_NC_CACHE = None


def _prep_inputs(x0: np.ndarray, linCombs: np.ndarray):
    """x0 [C,H,W] f32, linCombs [O, C*9] -> per-core in_maps."""
    Weff = _fold_weights(linCombs)
    wts_h = _build_block_weights(Weff)
    in_maps = []
    for core in range(NCORES):
        r0 = core * ROWS_PER_CORE
        ev = x0[:, r0:r0 + 2 * JT:2, :]       # [64, 33, W] even local rows
        od = x0[:, r0 + 1:r0 + 2 * JT:2, :]   # [64, 33, W] odd local rows
        P = np.ascontiguousarray(
            np.concatenate([ev, od], axis=0).astype(np.float16))
        in_maps.append({"xin": P, "wts": wts_h})
    return in_maps


def _decode_output(results) -> np.ndarray:
    out = np.empty((1, O, HO, WO), dtype=np.float32)
    for core in range(NCORES):
        y = np.asarray(results[core]["yout"], dtype=np.float32)
        # [NG, 128, 2*GSIZE*WO] -> [g, par(2), o(64), tloc, v, w]
        y = y.reshape(NG, 2, O, GSIZE, 2, WO)
        r0 = core * ROWS_PER_CORE
        for tloc in range(GSIZE):
            for v in range(2):
                for par in range(2):
                    row_off = 4 * tloc + 2 * v + (par if v == 0 else 1 - par)
                    out[0, :, r0 + row_off : r0 + ROWS_PER_CORE : 4 * GSIZE, :] = (
                        y[:, par, :, tloc, v, :].transpose(1, 0, 2))
    return out


def kernel(input: np.ndarray, linCombs: np.ndarray) -> np.ndarray:
    global _NC_CACHE
    x = np.ascontiguousarray(np.asarray(input, dtype=np.float32))
    L = np.asarray(linCombs, dtype=np.float32)
    assert x.shape == (1, C, H, W), x.shape

    in_maps = _prep_inputs(x[0], L)
    if _NC_CACHE is None:
        _NC_CACHE = _build_program()
    res = run_bass_kernel_spmd(_NC_CACHE, in_maps, list(range(NCORES)))
    return _decode_output(res.results)


# revision 10
# speedup vs baseline: 1.1188x; 1.0104x over previous
"""GCK 3x3 conv layer (nn_GCK3x3Layer) as a Trainium2 Bass kernel on 8 NeuronCores.

Math: out[o,h,w] = sum_{c,r,s} Weff[o,c,r,s] * x[c,h+r,w+s], where Weff is the
GCK linComb folded back through the +/-1 separable basis (done on host in f64).

Sharding: H split across 8 cores (64 output rows each, 66 input rows with halo).

Device scheme (col-tiled concurrent streams): input rows stored as even/odd
64-partition planes at the same free index.  Per row pair (p normal / p+1
psum-flipped), "main" K=128/M=64 matmuls for two output rows run pairwise
CONCURRENT in opposite PE column groups, and the four K=64 leftover-tap matmuls
fill all four 64x64 array quadrants concurrently: 9 wall slots of 512 cycles
per 2 row pairs (100% PE-array utilization at fp16).

Weight-group schedule: groups of 2 pair-pairs (4 psum banks) run all main
slots, then all leftover slots (ABBA phase order across groups), so the
(128,64)<->(64,64) tile-config switch (~120ns; LDWEIGHTS hides in steady state
but config changes do not) is paid once per group.  8-bank psum rotation lets
group g+1 accumulate while group g drains via DVE+ACT copies (fp32->fp16) to
SBUF, one 4KB-per-partition-descriptor DMA per group.  The last two groups are
single-t with leftovers-first so the final copies overlap the final mains.

Head: weights + the first 3 input row-pair slots ship as ONE leading DMA on
the sync ring (single gate semaphore for the first real matmul); remaining
input follows in ~0.65us-apart chunks (HWDGE descriptor-gen serializes per
ring, and SDMA engines drain packet-granular, so order = priority).  Warmup
matmuls on a gpsimd-zeroed tile lift the PE HAM clock gate (1.2->2.4 GHz)
during the DMA window.  Outputs ride the scalar ring.

Compute dtype: float16; fp16 output staging halves output HBM traffic
(rel err ~3.4e-4 on the graded seed-0 inputs).
"""

import numpy as np

import concourse.bass as bass
import concourse.mybir as mybir
import concourse.tile as tile
from concourse import bacc
from concourse.bass_utils import run_bass_kernel_spmd

# Problem constants (hardcoded per contract)
C = 64          # input channels
O = 64          # output channels
H = W = 514     # input spatial
HO = WO = 512   # output spatial
NCORES = 8
ROWS_PER_CORE = HO // NCORES          # 64 output rows
PAIRS = ROWS_PER_CORE // 2            # 32 row pairs
JT = ROWS_PER_CORE // 2 + 1           # 33 input row-pair slots (incl. halo)
NT = PAIRS // 2                       # 16 pair-pairs ("t" units, 4 rows each)
# (t_start, n_t) weight-groups: 7x G=2, then 2x G=1 to shorten the tail
GROUPS = [(2 * i, 2) for i in range(7)] + [(14, 1), (15, 1)]
WCOLS = 9 * 64                        # weight columns prepended to chunk 0
# Input chunks (row-pair slots per dma_start); chunk 0 also carries weights.
XGS = [3, 3, 4, 5, 6, 6, 6]           # sums to 33
XGO = [sum(XGS[:i]) for i in range(len(XGS))]
NWARM = 6                             # dummy matmuls to lift the PE HAM clock
                                      # gate (1.2->2.4 GHz) during input DMA

V = np.array([[1.0, 1.0, 1.0], [1.0, -1.0, 1.0], [1.0, 1.0, -1.0]], dtype=np.float64)

MM_DT = mybir.dt.float16   # matmul operand dtype
OUT_DT = mybir.dt.float16  # output staging dtype (cast back to f32 on host)


def _fold_weights(linCombs: np.ndarray) -> np.ndarray:
    """linCombs (O, C*9) -> effective conv kernels Weff (O, C, 3, 3), f64."""
    L = linCombs.astype(np.float64).reshape(O, C, 3, 3)  # k = c*9 + 3i + j
    return np.einsum("ocij,ir,js->ocrs", L, V, V)


def _build_block_weights(Weff: np.ndarray) -> np.ndarray:
    """Weights for the col-tiled scheme, returned as [128, 9*64] (k, idx*64+mu).

    idx 0..2  (s): K0=Wt(0,s), K1=Wt(1,s)  -- mains for EVEN output rows (rhs slot p)
    idx 3..5  (s): K0=Wt(1,s), K1=Wt(2,s)  -- mains for ODD  output rows (rhs slot p+1)
    idx 6..8  (s): K0=Wt(2,s) (leftover r2, even-plane rhs, partitions 0-63)
                   K1=Wt(0,s) (leftover r0, odd-plane rhs,  partitions 64-127)
    """
    Wt = {(r, s): Weff[:, :, r, s].T for r in range(3) for s in range(3)}  # [c, o]
    mats = np.zeros((9, 128, 64), dtype=np.float64)
    for s in range(3):
        mats[s, 0:64] = Wt[(0, s)]
        mats[s, 64:128] = Wt[(1, s)]
        mats[3 + s, 0:64] = Wt[(1, s)]
        mats[3 + s, 64:128] = Wt[(2, s)]
        mats[6 + s, 0:64] = Wt[(2, s)]
        mats[6 + s, 64:128] = Wt[(0, s)]
    m = mats.transpose(1, 0, 2).reshape(128, 9 * 64)
    return np.ascontiguousarray(m.astype(np.float16))


def _build_program():
    nc = bacc.Bacc(None, target_bir_lowering=False, enable_partition_id=False)
    # xin[:, 0:WCOLS] = block weights; xin[:, WCOLS + j*W ...] = row-pair slot j
    xin = nc.declare_dram_parameter(
        "xin", [128, WCOLS + JT * W], MM_DT, isOutput=False
    )
    yout = nc.declare_dram_parameter(
        "yout", [128, PAIRS * WO], OUT_DT, isOutput=True
    )

    with tile.TileContext(nc) as tc:
        with (
            tc.tile_pool(name="wpool", bufs=1) as wpool,
            tc.tile_pool(name="xpool", bufs=1) as xpool,
            tc.tile_pool(name="opool", bufs=3) as opool,
            tc.tile_pool(name="pspool", bufs=8, space="PSUM") as pspool,
        ):
            warm = wpool.tile([128, WO], mybir.dt.bfloat16, name="warm")
            nc.gpsimd.memset(warm[:], 0.0)
            # preload the ACT table off the critical path (first use ~1.3us)
            actw = wpool.tile([128, 16], mybir.dt.float32, name="actw")
            nc.scalar.copy(actw[:], warm[:, 0:16])
            wps = pspool.tile([128, WO], mybir.dt.float32, tag="ps", name="wps")
            for _ in range(NWARM):
                nc.tensor.matmul(
                    wps[:], warm[:, :128], warm[:], start=True, stop=True
                )

            xgs = []
            for gx, n in enumerate(XGS):
                ecols = (WCOLS if gx == 0 else 0) + n * W
                off = 0 if gx == 0 else WCOLS + XGO[gx] * W
                xt = xpool.tile([128, ecols], MM_DT, tag=f"xt{gx}", name=f"xt{gx}")
                nc.sync.dma_start(xt[:], xin[:, off : off + ecols])
                xgs.append((XGO[gx], n, xt))

            wt0 = xgs[0][2]  # chunk 0 tile; first WCOLS columns are weights

            def wtm(idx):            # main weight block [128, 64]
                return wt0[:, idx * 64 : (idx + 1) * 64]

            def rhs(j, s):
                for start, n, xt in reversed(xgs):
                    if j >= start:
                        off = (WCOLS if start == 0 else 0) + (j - start) * W + s
                        return xt[:, off : off + WO]
                raise AssertionError(j)

            for g, (t0, gn) in enumerate(GROUPS):
                ts = [t0 + i for i in range(gn)]
                ps = [
                    [
                        pspool.tile([128, WO], mybir.dt.float32, tag="ps",
                                    name=f"ps{t}_{v}")
                        for v in range(2)
                    ]
                    for t in ts
                ]
                # ABBA phase order (last group leftovers-first so its final
                # copies overlap the final main slots).
                lo_first = (g % 2 == 1) or g == len(GROUPS) - 1

                def mains(first, last):
                    for i, t in enumerate(ts):
                        p0, p1 = 2 * t, 2 * t + 1
                        for s in range(3):
                            st, sp = first and s == 0, last and s == 2
                            nc.tensor.matmul(ps[i][0][0:64, :], wtm(s),
                                             rhs(p0, s), start=st, stop=sp)
                            nc.tensor.matmul(ps[i][0][64:128, :], wtm(3 + s),
                                             rhs(p0 + 1, s), start=st, stop=sp)
                        for s in range(3):
                            st, sp = first and s == 0, last and s == 2
                            nc.tensor.matmul(ps[i][1][64:128, :], wtm(s),
                                             rhs(p1, s), start=st, stop=sp)
                            nc.tensor.matmul(ps[i][1][0:64, :], wtm(3 + s),
                                             rhs(p1 + 1, s), start=st, stop=sp)

                def leftovers(first, last):
                    for s in range(3):
                        st, sp = first and s == 0, last and s == 2
                        for i, t in enumerate(ts):
                            p0, p1 = 2 * t, 2 * t + 1
                            e0 = rhs(p0 + 1, s)  # even half -> partitions 0-63
                            o0 = rhs(p0, s)      # odd half  -> partitions 64-127
                            e1 = rhs(p1 + 1, s)
                            o1 = rhs(p1, s)
                            w_e = wtm(6 + s)[0:64, :]
                            w_o = wtm(6 + s)[64:128, :]
                            nc.tensor.matmul(ps[i][0][0:64, :], w_e,
                                             e0[0:64, :], start=st, stop=sp)
                            nc.tensor.matmul(ps[i][0][64:128, :], w_o,
                                             o0[64:128, :], start=st, stop=sp)
                            nc.tensor.matmul(ps[i][1][64:128, :], w_e,
                                             e1[0:64, :], start=st, stop=sp)
                            nc.tensor.matmul(ps[i][1][0:64, :], w_o,
                                             o1[64:128, :], start=st, stop=sp)

                if lo_first:
                    leftovers(True, False)
                    mains(False, True)
                else:
                    mains(True, False)
                    leftovers(False, True)

                # drain: fp32->fp16 copies split across DVE and ACT, then one
                # output DMA (last group: split per copy across both rings).
                ot = opool.tile([128, 2 * gn * WO], OUT_DT, tag="ot", name=f"ot{g}")
                last = g == len(GROUPS) - 1
                colbase = 2 * t0 * WO
                for i in range(gn):
                    nc.vector.tensor_copy(
                        ot[:, (2 * i) * WO : (2 * i + 1) * WO], ps[i][0][:])
                    nc.scalar.copy(
                        ot[:, (2 * i + 1) * WO : (2 * i + 2) * WO], ps[i][1][:])
                if last:
                    nc.sync.dma_start(
                        yout[:, colbase : colbase + WO], ot[:, 0:WO])
                    nc.scalar.dma_start(
                        yout[:, colbase + WO : colbase + 2 * WO], ot[:, WO : 2 * WO])
                else:
                    nc.scalar.dma_start(
                        yout[:, colbase : colbase + 2 * gn * WO], ot[:])

    nc.compile()
    return nc


_NC_CACHE = None


def _prep_inputs(x0: np.ndarray, linCombs: np.ndarray):
    """x0 [C,H,W] f32, linCombs [O, C*9] -> per-core in_maps."""
    Weff = _fold_weights(linCombs)
    wts_h = _build_block_weights(Weff)           # [128, 576] fp16
    in_maps = []
    for core in range(NCORES):
        r0 = core * ROWS_PER_CORE
        ev = x0[:, r0:r0 + 2 * JT:2, :]          # [64, 33, W] even local rows
        od = x0[:, r0 + 1:r0 + 2 * JT:2, :]      # [64, 33, W] odd local rows
        P = np.concatenate([ev, od], axis=0).astype(np.float16).reshape(128, JT * W)
        buf = np.empty((128, WCOLS + JT * W), dtype=np.float16)
        buf[:, :WCOLS] = wts_h
        buf[:, WCOLS:] = P
        in_maps.append({"xin": buf})
    return in_maps


def _decode_output(results) -> np.ndarray:
    out = np.empty((1, O, HO, WO), dtype=np.float32)
    for core in range(NCORES):
        y = np.asarray(results[core]["yout"], dtype=np.float32)
        y = y.reshape(2, O, PAIRS, WO)           # [par, o, pair, w]
        r0 = core * ROWS_PER_CORE
        for par in range(2):
            # even pairs (ps[t][0], "normal"): row = 2p + par
            out[0, :, r0 + 0 + par : r0 + ROWS_PER_CORE : 4, :] = y[par, :, 0::2, :]
            # odd pairs (ps[t][1], flipped): row = 2p + 1 - par
            out[0, :, r0 + 2 + 1 - par : r0 + ROWS_PER_CORE : 4, :] = y[par, :, 1::2, :]
    return out


def kernel(input: np.ndarray, linCombs: np.ndarray) -> np.ndarray:
    global _NC_CACHE
    x = np.ascontiguousarray(np.asarray(input, dtype=np.float32))
    L = np.asarray(linCombs, dtype=np.float32)
    assert x.shape == (1, C, H, W), x.shape

    in_maps = _prep_inputs(x[0], L)
    if _NC_CACHE is None:
        _NC_CACHE = _build_program()
    res = run_bass_kernel_spmd(_NC_CACHE, in_maps, list(range(NCORES)))
    return _decode_output(res.results)


# revision 12
# speedup vs baseline: 1.1411x; 1.0199x over previous
"""GCK 3x3 conv layer (nn_GCK3x3Layer) as a Trainium2 Bass kernel on 8 NeuronCores.

Math: out[o,h,w] = sum_{c,r,s} Weff[o,c,r,s] * x[c,h+r,w+s], where Weff is the
GCK linComb folded back through the +/-1 separable basis (done on host in f64).

Sharding: H split across 8 cores (64 output rows each, 66 input rows with halo).

Device scheme (col-tiled concurrent streams): input rows stored as even/odd
64-partition planes at the same free index.  Per row pair (p normal / p+1
psum-flipped), "main" K=128/M=64 matmuls for two output rows run pairwise
CONCURRENT in opposite PE column groups, and the four K=64 leftover-tap matmuls
fill all four 64x64 array quadrants concurrently: 9 wall slots of 512 cycles
per 2 row pairs (100% PE-array utilization at fp16).

Weight-group schedule: groups of 2 pair-pairs (4 psum banks) run all main
slots, then all leftover slots (ABBA phase order across groups), so the
(128,64)<->(64,64) tile-config switch (~120ns; LDWEIGHTS hides in steady state
but config changes do not) is paid once per group.  8-bank psum rotation lets
group g+1 accumulate while group g drains via DVE+ACT copies (fp32->fp16) to
SBUF, one 4KB-per-partition-descriptor DMA per group.  The last two groups are
single-t with leftovers-first so the final copies overlap the final mains.

Head: weights + the first 3 input row-pair slots ship as ONE leading DMA on
the sync ring (single gate semaphore for the first real matmul); remaining
input follows in ~0.65us-apart chunks (HWDGE descriptor-gen serializes per
ring, and SDMA engines drain packet-granular, so order = priority).  Warmup
matmuls on a gpsimd-zeroed tile lift the PE HAM clock gate (1.2->2.4 GHz)
during the DMA window.  Outputs ride the scalar ring.

Compute dtype: float16; fp16 output staging halves output HBM traffic
(rel err ~3.4e-4 on the graded seed-0 inputs).
"""

import numpy as np

import concourse.bass as bass
import concourse.mybir as mybir
import concourse.tile as tile
from concourse import bacc
from concourse.bass_utils import run_bass_kernel_spmd

# Problem constants (hardcoded per contract)
C = 64          # input channels
O = 64          # output channels
H = W = 514     # input spatial
HO = WO = 512   # output spatial
NCORES = 8
ROWS_PER_CORE = HO // NCORES          # 64 output rows
PAIRS = ROWS_PER_CORE // 2            # 32 row pairs
JT = ROWS_PER_CORE // 2 + 1           # 33 input row-pair slots (incl. halo)
NT = PAIRS // 2                       # 16 pair-pairs ("t" units, 4 rows each)
# (t_start, n_t) weight-groups: 7x G=2, then 2x G=1 to shorten the tail
GROUPS = [(2 * i, 2) for i in range(7)] + [(14, 1), (15, 1)]
WCOLS = 9 * 64                        # weight columns prepended to chunk 0
# Input chunks (row-pair slots per dma_start); chunk 0 also carries weights
# and stays under one 4KB descriptor per partition for the fastest gate.
XGS = [2, 3, 4, 5, 6, 6, 7]           # sums to 33
XGO = [sum(XGS[:i]) for i in range(len(XGS))]
NWARM = 15                            # N=256 dummy matmuls (~213ns cold each,
                                      # ~3.2us total) lifting the PE HAM clock
                                      # gate (1.2->2.4 GHz); sized to end just
                                      # AFTER the chunk-0 semaphore on every
                                      # core -- a PE idle gap between warmups
                                      # and the first real matmul resets the
                                      # HAM activity window and costs ~2-3us

V = np.array([[1.0, 1.0, 1.0], [1.0, -1.0, 1.0], [1.0, 1.0, -1.0]], dtype=np.float64)

MM_DT = mybir.dt.float16   # matmul operand dtype
OUT_DT = mybir.dt.float16  # output staging dtype (cast back to f32 on host)


def _fold_weights(linCombs: np.ndarray) -> np.ndarray:
    """linCombs (O, C*9) -> effective conv kernels Weff (O, C, 3, 3), f64."""
    L = linCombs.astype(np.float64).reshape(O, C, 3, 3)  # k = c*9 + 3i + j
    return np.einsum("ocij,ir,js->ocrs", L, V, V)


def _build_block_weights(Weff: np.ndarray) -> np.ndarray:
    """Weights for the col-tiled scheme, returned as [128, 9*64] (k, idx*64+mu).

    idx 0..2  (s): K0=Wt(0,s), K1=Wt(1,s)  -- mains for EVEN output rows (rhs slot p)
    idx 3..5  (s): K0=Wt(1,s), K1=Wt(2,s)  -- mains for ODD  output rows (rhs slot p+1)
    idx 6..8  (s): K0=Wt(2,s) (leftover r2, even-plane rhs, partitions 0-63)
                   K1=Wt(0,s) (leftover r0, odd-plane rhs,  partitions 64-127)
    """
    Wt = {(r, s): Weff[:, :, r, s].T for r in range(3) for s in range(3)}  # [c, o]
    mats = np.zeros((9, 128, 64), dtype=np.float64)
    for s in range(3):
        mats[s, 0:64] = Wt[(0, s)]
        mats[s, 64:128] = Wt[(1, s)]
        mats[3 + s, 0:64] = Wt[(1, s)]
        mats[3 + s, 64:128] = Wt[(2, s)]
        mats[6 + s, 0:64] = Wt[(2, s)]
        mats[6 + s, 64:128] = Wt[(0, s)]
    m = mats.transpose(1, 0, 2).reshape(128, 9 * 64)
    return np.ascontiguousarray(m.astype(np.float16))


def _build_program():
    nc = bacc.Bacc(None, target_bir_lowering=False, enable_partition_id=False)
    # xin[:, 0:WCOLS] = block weights; xin[:, WCOLS + j*W ...] = row-pair slot j
    xin = nc.declare_dram_parameter(
        "xin", [128, WCOLS + JT * W], MM_DT, isOutput=False
    )
    yout = nc.declare_dram_parameter(
        "yout", [128, PAIRS * WO], OUT_DT, isOutput=True
    )

    with tile.TileContext(nc) as tc:
        with (
            tc.tile_pool(name="wpool", bufs=1) as wpool,
            tc.tile_pool(name="xpool", bufs=1) as xpool,
            tc.tile_pool(name="opool", bufs=3) as opool,
            tc.tile_pool(name="pspool", bufs=8, space="PSUM") as pspool,
        ):
            warm = wpool.tile([128, WO], mybir.dt.bfloat16, name="warm")
            nc.gpsimd.memset(warm[:], 0.0)
            # preload the ACT table off the critical path (first use ~1.3us)
            actw = wpool.tile([128, 16], mybir.dt.float32, name="actw")
            nc.scalar.copy(actw[:], warm[:, 0:16])
            wps = pspool.tile([128, WO], mybir.dt.float32, tag="ps", name="wps")
            for _ in range(NWARM):
                nc.tensor.matmul(
                    wps[:, 0:256], warm[:, :128], warm[:, 0:256],
                    start=True, stop=True
                )

            xgs = []
            for gx, n in enumerate(XGS):
                ecols = (WCOLS if gx == 0 else 0) + n * W
                off = 0 if gx == 0 else WCOLS + XGO[gx] * W
                xt = xpool.tile([128, ecols], MM_DT, tag=f"xt{gx}", name=f"xt{gx}")
                nc.sync.dma_start(xt[:], xin[:, off : off + ecols])
                xgs.append((XGO[gx], n, xt))

            wt0 = xgs[0][2]  # chunk 0 tile; first WCOLS columns are weights

            def wtm(idx):            # main weight block [128, 64]
                return wt0[:, idx * 64 : (idx + 1) * 64]

            def rhs(j, s):
                for start, n, xt in reversed(xgs):
                    if j >= start:
                        off = (WCOLS if start == 0 else 0) + (j - start) * W + s
                        return xt[:, off : off + WO]
                raise AssertionError(j)

            for g, (t0, gn) in enumerate(GROUPS):
                ts = [t0 + i for i in range(gn)]
                ps = [
                    [
                        pspool.tile([128, WO], mybir.dt.float32, tag="ps",
                                    name=f"ps{t}_{v}")
                        for v in range(2)
                    ]
                    for t in ts
                ]
                # ABBA phase order (last group leftovers-first so its final
                # copies overlap the final main slots).
                lo_first = (g % 2 == 1) or g == len(GROUPS) - 1

                def mains(first, last):
                    for i, t in enumerate(ts):
                        p0, p1 = 2 * t, 2 * t + 1
                        for s in range(3):
                            st, sp = first and s == 0, last and s == 2
                            nc.tensor.matmul(ps[i][0][0:64, :], wtm(s),
                                             rhs(p0, s), start=st, stop=sp)
                            nc.tensor.matmul(ps[i][0][64:128, :], wtm(3 + s),
                                             rhs(p0 + 1, s), start=st, stop=sp)
                        for s in range(3):
                            st, sp = first and s == 0, last and s == 2
                            nc.tensor.matmul(ps[i][1][64:128, :], wtm(s),
                                             rhs(p1, s), start=st, stop=sp)
                            nc.tensor.matmul(ps[i][1][0:64, :], wtm(3 + s),
                                             rhs(p1 + 1, s), start=st, stop=sp)

                def leftovers(first, last):
                    for s in range(3):
                        st, sp = first and s == 0, last and s == 2
                        for i, t in enumerate(ts):
                            p0, p1 = 2 * t, 2 * t + 1
                            e0 = rhs(p0 + 1, s)  # even half -> partitions 0-63
                            o0 = rhs(p0, s)      # odd half  -> partitions 64-127
                            e1 = rhs(p1 + 1, s)
                            o1 = rhs(p1, s)
                            w_e = wtm(6 + s)[0:64, :]
                            w_o = wtm(6 + s)[64:128, :]
                            nc.tensor.matmul(ps[i][0][0:64, :], w_e,
                                             e0[0:64, :], start=st, stop=sp)
                            nc.tensor.matmul(ps[i][0][64:128, :], w_o,
                                             o0[64:128, :], start=st, stop=sp)
                            nc.tensor.matmul(ps[i][1][64:128, :], w_e,
                                             e1[0:64, :], start=st, stop=sp)
                            nc.tensor.matmul(ps[i][1][0:64, :], w_o,
                                             o1[64:128, :], start=st, stop=sp)

                if lo_first:
                    leftovers(True, False)
                    mains(False, True)
                else:
                    mains(True, False)
                    leftovers(False, True)

                # drain: fp32->fp16 copies split across DVE and ACT, then one
                # output DMA (last group: split per copy across both rings).
                ot = opool.tile([128, 2 * gn * WO], OUT_DT, tag="ot", name=f"ot{g}")
                last = g == len(GROUPS) - 1
                colbase = 2 * t0 * WO
                for i in range(gn):
                    nc.vector.tensor_copy(
                        ot[:, (2 * i) * WO : (2 * i + 1) * WO], ps[i][0][:])
                    nc.scalar.copy(
                        ot[:, (2 * i + 1) * WO : (2 * i + 2) * WO], ps[i][1][:])
                if last:
                    nc.sync.dma_start(
                        yout[:, colbase : colbase + WO], ot[:, 0:WO])
                    nc.scalar.dma_start(
                        yout[:, colbase + WO : colbase + 2 * WO], ot[:, WO : 2 * WO])
                else:
                    nc.scalar.dma_start(
                        yout[:, colbase : colbase + 2 * gn * WO], ot[:])

    nc.compile()
    return nc


_NC_CACHE = None


def _prep_inputs(x0: np.ndarray, linCombs: np.ndarray):
    """x0 [C,H,W] f32, linCombs [O, C*9] -> per-core in_maps."""
    Weff = _fold_weights(linCombs)
    wts_h = _build_block_weights(Weff)           # [128, 576] fp16
    in_maps = []
    for core in range(NCORES):
        r0 = core * ROWS_PER_CORE
        ev = x0[:, r0:r0 + 2 * JT:2, :]          # [64, 33, W] even local rows
        od = x0[:, r0 + 1:r0 + 2 * JT:2, :]      # [64, 33, W] odd local rows
        P = np.concatenate([ev, od], axis=0).astype(np.float16).reshape(128, JT * W)
        buf = np.empty((128, WCOLS + JT * W), dtype=np.float16)
        buf[:, :WCOLS] = wts_h
        buf[:, WCOLS:] = P
        in_maps.append({"xin": buf})
    return in_maps


def _decode_output(results) -> np.ndarray:
    out = np.empty((1, O, HO, WO), dtype=np.float32)
    for core in range(NCORES):
        y = np.asarray(results[core]["yout"], dtype=np.float32)
        y = y.reshape(2, O, PAIRS, WO)           # [par, o, pair, w]
        r0 = core * ROWS_PER_CORE
        for par in range(2):
            # even pairs (ps[t][0], "normal"): row = 2p + par
            out[0, :, r0 + 0 + par : r0 + ROWS_PER_CORE : 4, :] = y[par, :, 0::2, :]
            # odd pairs (ps[t][1], flipped): row = 2p + 1 - par
            out[0, :, r0 + 2 + 1 - par : r0 + ROWS_PER_CORE : 4, :] = y[par, :, 1::2, :]
    return out


def kernel(input: np.ndarray, linCombs: np.ndarray) -> np.ndarray:
    global _NC_CACHE
    x = np.ascontiguousarray(np.asarray(input, dtype=np.float32))
    L = np.asarray(linCombs, dtype=np.float32)
    assert x.shape == (1, C, H, W), x.shape

    in_maps = _prep_inputs(x[0], L)
    if _NC_CACHE is None:
        _NC_CACHE = _build_program()
    res = run_bass_kernel_spmd(_NC_CACHE, in_maps, list(range(NCORES)))
    return _decode_output(res.results)


# revision 13
# speedup vs baseline: 1.1516x; 1.0092x over previous
"""GCK 3x3 conv layer (nn_GCK3x3Layer) as a Trainium2 Bass kernel on 8 NeuronCores.

Math: out[o,h,w] = sum_{c,r,s} Weff[o,c,r,s] * x[c,h+r,w+s], where Weff is the
GCK linComb folded back through the +/-1 separable basis (done on host in f64).

Sharding: H split across 8 cores (64 output rows each, 66 input rows with halo).

Device scheme (col-tiled concurrent streams): input rows stored as even/odd
64-partition planes at the same free index.  Per row pair (p normal / p+1
psum-flipped), "main" K=128/M=64 matmuls for two output rows run pairwise
CONCURRENT in opposite PE column groups, and the four K=64 leftover-tap matmuls
fill all four 64x64 array quadrants concurrently: 9 wall slots of 512 cycles
per 2 row pairs (100% PE-array utilization at fp16).

Weight-group schedule: groups of 2 pair-pairs (4 psum banks) run all main
slots, then all leftover slots (ABBA phase order across groups), so the
(128,64)<->(64,64) tile-config switch (~120ns; LDWEIGHTS hides in steady state
but config changes do not) is paid once per group.  8-bank psum rotation lets
group g+1 accumulate while group g drains via DVE+ACT copies (fp32->fp16) to
SBUF, one 4KB-per-partition-descriptor DMA per group.  The last two groups are
single-t with leftovers-first so the final copies overlap the final mains.

Head: weights + the first 3 input row-pair slots ship as ONE leading DMA on
the sync ring (single gate semaphore for the first real matmul); remaining
input follows in ~0.65us-apart chunks (HWDGE descriptor-gen serializes per
ring, and SDMA engines drain packet-granular, so order = priority).  Warmup
matmuls on a gpsimd-zeroed tile lift the PE HAM clock gate (1.2->2.4 GHz)
during the DMA window.  Outputs ride the scalar ring.

Compute dtype: float16; fp16 output staging halves output HBM traffic
(rel err ~3.4e-4 on the graded seed-0 inputs).
"""

import numpy as np

import concourse.bass as bass
import concourse.mybir as mybir
import concourse.tile as tile
from concourse import bacc
from concourse.bass_utils import run_bass_kernel_spmd

# Problem constants (hardcoded per contract)
C = 64          # input channels
O = 64          # output channels
H = W = 514     # input spatial
HO = WO = 512   # output spatial
NCORES = 8
ROWS_PER_CORE = HO // NCORES          # 64 output rows
PAIRS = ROWS_PER_CORE // 2            # 32 row pairs
JT = ROWS_PER_CORE // 2 + 1           # 33 input row-pair slots (incl. halo)
NT = PAIRS // 2                       # 16 pair-pairs ("t" units, 4 rows each)
# (t_start, n_t) weight-groups: two single-t groups first (their 9-slot span
# consumes input as slowly as the chunk pipeline delivers it early on), G=2 in
# the middle, two single-t groups last to shorten the drain tail.
GROUPS = ([(0, 1), (1, 1)] + [(2 + 2 * i, 2) for i in range(6)]
          + [(14, 1), (15, 1)])
WCOLS = 9 * 64                        # weight columns prepended to chunk 0
# Input chunks (row-pair slots per dma_start); chunk 0 carries the weights
# plus all three slots group (0,1) touches, so one semaphore gates the start.
XGS = [3, 2, 4, 5, 6, 6, 7]           # sums to 33
XGO = [sum(XGS[:i]) for i in range(len(XGS))]
NWARM = 19                            # N=256 dummy matmuls (213ns cold each,
                                      # ~4us total) lifting the PE HAM clock
                                      # gate (1.2->2.4 GHz); sized to end just
                                      # AFTER the chunk-0 semaphore on every
                                      # core -- a PE idle gap BEFORE the gate
                                      # lifts resets the HAM activity window
                                      # and costs ~2-3us (gaps after are free)

V = np.array([[1.0, 1.0, 1.0], [1.0, -1.0, 1.0], [1.0, 1.0, -1.0]], dtype=np.float64)

MM_DT = mybir.dt.float16   # matmul operand dtype
OUT_DT = mybir.dt.float16  # output staging dtype (cast back to f32 on host)


def _fold_weights(linCombs: np.ndarray) -> np.ndarray:
    """linCombs (O, C*9) -> effective conv kernels Weff (O, C, 3, 3), f64."""
    L = linCombs.astype(np.float64).reshape(O, C, 3, 3)  # k = c*9 + 3i + j
    return np.einsum("ocij,ir,js->ocrs", L, V, V)


def _build_block_weights(Weff: np.ndarray) -> np.ndarray:
    """Weights for the col-tiled scheme, returned as [128, 9*64] (k, idx*64+mu).

    idx 0..2  (s): K0=Wt(0,s), K1=Wt(1,s)  -- mains for EVEN output rows (rhs slot p)
    idx 3..5  (s): K0=Wt(1,s), K1=Wt(2,s)  -- mains for ODD  output rows (rhs slot p+1)
    idx 6..8  (s): K0=Wt(2,s) (leftover r2, even-plane rhs, partitions 0-63)
                   K1=Wt(0,s) (leftover r0, odd-plane rhs,  partitions 64-127)
    """
    Wt = {(r, s): Weff[:, :, r, s].T for r in range(3) for s in range(3)}  # [c, o]
    mats = np.zeros((9, 128, 64), dtype=np.float64)
    for s in range(3):
        mats[s, 0:64] = Wt[(0, s)]
        mats[s, 64:128] = Wt[(1, s)]
        mats[3 + s, 0:64] = Wt[(1, s)]
        mats[3 + s, 64:128] = Wt[(2, s)]
        mats[6 + s, 0:64] = Wt[(2, s)]
        mats[6 + s, 64:128] = Wt[(0, s)]
    m = mats.transpose(1, 0, 2).reshape(128, 9 * 64)
    return np.ascontiguousarray(m.astype(np.float16))


def _build_program():
    nc = bacc.Bacc(None, target_bir_lowering=False, enable_partition_id=False)
    # xin[:, 0:WCOLS] = block weights; xin[:, WCOLS + j*W ...] = row-pair slot j
    xin = nc.declare_dram_parameter(
        "xin", [128, WCOLS + JT * W], MM_DT, isOutput=False
    )
    yout = nc.declare_dram_parameter(
        "yout", [128, PAIRS * WO], OUT_DT, isOutput=True
    )

    with tile.TileContext(nc) as tc:
        with (
            tc.tile_pool(name="wpool", bufs=1) as wpool,
            tc.tile_pool(name="xpool", bufs=1) as xpool,
            tc.tile_pool(name="opool", bufs=3) as opool,
            tc.tile_pool(name="pspool", bufs=8, space="PSUM") as pspool,
        ):
            warm = wpool.tile([128, WO], mybir.dt.bfloat16, name="warm")
            nc.gpsimd.memset(warm[:], 0.0)
            # preload the ACT table off the critical path (first use ~1.3us)
            actw = wpool.tile([128, 16], mybir.dt.float32, name="actw")
            nc.scalar.copy(actw[:], warm[:, 0:16])
            wps = pspool.tile([128, WO], mybir.dt.float32, tag="ps", name="wps")
            for _ in range(NWARM):
                nc.tensor.matmul(
                    wps[:, 0:256], warm[:, :128], warm[:, 0:256],
                    start=True, stop=True
                )

            xgs = []
            for gx, n in enumerate(XGS):
                ecols = (WCOLS if gx == 0 else 0) + n * W
                off = 0 if gx == 0 else WCOLS + XGO[gx] * W
                xt = xpool.tile([128, ecols], MM_DT, tag=f"xt{gx}", name=f"xt{gx}")
                nc.sync.dma_start(xt[:], xin[:, off : off + ecols])
                xgs.append((XGO[gx], n, xt))

            wt0 = xgs[0][2]  # chunk 0 tile; first WCOLS columns are weights

            def wtm(idx):            # main weight block [128, 64]
                return wt0[:, idx * 64 : (idx + 1) * 64]

            def rhs(j, s):
                for start, n, xt in reversed(xgs):
                    if j >= start:
                        off = (WCOLS if start == 0 else 0) + (j - start) * W + s
                        return xt[:, off : off + WO]
                raise AssertionError(j)

            for g, (t0, gn) in enumerate(GROUPS):
                ts = [t0 + i for i in range(gn)]
                ps = [
                    [
                        pspool.tile([128, WO], mybir.dt.float32, tag="ps",
                                    name=f"ps{t}_{v}")
                        for v in range(2)
                    ]
                    for t in ts
                ]
                # ABBA phase order (last group leftovers-first so its final
                # copies overlap the final main slots).
                lo_first = (g % 2 == 1) or g == len(GROUPS) - 1

                def mains(first, last):
                    for i, t in enumerate(ts):
                        p0, p1 = 2 * t, 2 * t + 1
                        for s in range(3):
                            st, sp = first and s == 0, last and s == 2
                            nc.tensor.matmul(ps[i][0][0:64, :], wtm(s),
                                             rhs(p0, s), start=st, stop=sp)
                            nc.tensor.matmul(ps[i][0][64:128, :], wtm(3 + s),
                                             rhs(p0 + 1, s), start=st, stop=sp)
                        for s in range(3):
                            st, sp = first and s == 0, last and s == 2
                            nc.tensor.matmul(ps[i][1][64:128, :], wtm(s),
                                             rhs(p1, s), start=st, stop=sp)
                            nc.tensor.matmul(ps[i][1][0:64, :], wtm(3 + s),
                                             rhs(p1 + 1, s), start=st, stop=sp)

                def leftovers(first, last):
                    for s in range(3):
                        st, sp = first and s == 0, last and s == 2
                        for i, t in enumerate(ts):
                            p0, p1 = 2 * t, 2 * t + 1
                            e0 = rhs(p0 + 1, s)  # even half -> partitions 0-63
                            o0 = rhs(p0, s)      # odd half  -> partitions 64-127
                            e1 = rhs(p1 + 1, s)
                            o1 = rhs(p1, s)
                            w_e = wtm(6 + s)[0:64, :]
                            w_o = wtm(6 + s)[64:128, :]
                            nc.tensor.matmul(ps[i][0][0:64, :], w_e,
                                             e0[0:64, :], start=st, stop=sp)
                            nc.tensor.matmul(ps[i][0][64:128, :], w_o,
                                             o0[64:128, :], start=st, stop=sp)
                            nc.tensor.matmul(ps[i][1][64:128, :], w_e,
                                             e1[0:64, :], start=st, stop=sp)
                            nc.tensor.matmul(ps[i][1][0:64, :], w_o,
                                             o1[64:128, :], start=st, stop=sp)

                if lo_first:
                    leftovers(True, False)
                    mains(False, True)
                else:
                    mains(True, False)
                    leftovers(False, True)

                # drain: fp32->fp16 copies split across DVE and ACT, then one
                # output DMA (last group: split per copy across both rings).
                ot = opool.tile([128, 2 * gn * WO], OUT_DT, tag="ot", name=f"ot{g}")
                last = g == len(GROUPS) - 1
                colbase = 2 * t0 * WO
                for i in range(gn):
                    nc.vector.tensor_copy(
                        ot[:, (2 * i) * WO : (2 * i + 1) * WO], ps[i][0][:])
                    nc.scalar.copy(
                        ot[:, (2 * i + 1) * WO : (2 * i + 2) * WO], ps[i][1][:])
                if last:
                    nc.sync.dma_start(
                        yout[:, colbase : colbase + WO], ot[:, 0:WO])
                    nc.scalar.dma_start(
                        yout[:, colbase + WO : colbase + 2 * WO], ot[:, WO : 2 * WO])
                else:
                    nc.scalar.dma_start(
                        yout[:, colbase : colbase + 2 * gn * WO], ot[:])

    nc.compile()
    return nc


_NC_CACHE = None


def _prep_inputs(x0: np.ndarray, linCombs: np.ndarray):
    """x0 [C,H,W] f32, linCombs [O, C*9] -> per-core in_maps."""
    Weff = _fold_weights(linCombs)
    wts_h = _build_block_weights(Weff)           # [128, 576] fp16
    in_maps = []
    for core in range(NCORES):
        r0 = core * ROWS_PER_CORE
        ev = x0[:, r0:r0 + 2 * JT:2, :]          # [64, 33, W] even local rows
        od = x0[:, r0 + 1:r0 + 2 * JT:2, :]      # [64, 33, W] odd local rows
        P = np.concatenate([ev, od], axis=0).astype(np.float16).reshape(128, JT * W)
        buf = np.empty((128, WCOLS + JT * W), dtype=np.float16)
        buf[:, :WCOLS] = wts_h
        buf[:, WCOLS:] = P
        in_maps.append({"xin": buf})
    return in_maps


def _decode_output(results) -> np.ndarray:
    out = np.empty((1, O, HO, WO), dtype=np.float32)
    for core in range(NCORES):
        y = np.asarray(results[core]["yout"], dtype=np.float32)
        y = y.reshape(2, O, PAIRS, WO)           # [par, o, pair, w]
        r0 = core * ROWS_PER_CORE
        for par in range(2):
            # even pairs (ps[t][0], "normal"): row = 2p + par
            out[0, :, r0 + 0 + par : r0 + ROWS_PER_CORE : 4, :] = y[par, :, 0::2, :]
            # odd pairs (ps[t][1], flipped): row = 2p + 1 - par
            out[0, :, r0 + 2 + 1 - par : r0 + ROWS_PER_CORE : 4, :] = y[par, :, 1::2, :]
    return out


def kernel(input: np.ndarray, linCombs: np.ndarray) -> np.ndarray:
    global _NC_CACHE
    x = np.ascontiguousarray(np.asarray(input, dtype=np.float32))
    L = np.asarray(linCombs, dtype=np.float32)
    assert x.shape == (1, C, H, W), x.shape

    in_maps = _prep_inputs(x[0], L)
    if _NC_CACHE is None:
        _NC_CACHE = _build_program()
    res = run_bass_kernel_spmd(_NC_CACHE, in_maps, list(range(NCORES)))
    return _decode_output(res.results)
